# revision 9
# baseline (speedup 1.0000x reference)
"""Trainium2 Bass kernel for nn_AttentionConvolution (graph attention layer).

B=8 graphs are data-parallel across the 8 NeuronCores: each core computes one
full graph's attention layer.  Inside a core (N=1024 tokens, D=1024, H=16
heads of DH=64):

  - hid is PE-transposed to feature-major hidT (bf16) once.
  - Wq/Wk/Wv stream in as f32 and are cast to bf16 (1/sqrt(D) folded into Wq).
  - The 6-entry rel_table gather is evaluated as the exact degree-5
    interpolating polynomial of relative_pos via a factored-Horner chain of
    scalar_tensor_tensor ops on the vector engine; the adjacency mask is
    folded in as an additive -60 logit penalty, all in one fused pipeline.
    The combined bias B is cast to bf16 and PE-transposed into B^T.
  - Scores are computed transposed (S^T[m,n], keys on partitions) so that the
    attention matrix comes out in the layout the A@V matmul needs.  B^T is
    injected into the PSUM accumulation with an identity matmul, so the exp
    activation reads PSUM once and applies no further masking.  Logits are
    bounded (~|3.5|) so softmax needs no max subtraction; the denominator is
    obtained for free by augmenting V with a ones column.
  - out^T[65,1024] per head (64 channels + denominator row) is PE-transposed
    back to token-major, divided + relu'd in one fused tensor_scalar, residual
    added, and layer-normed via bn_stats/bn_aggr.
"""

import numpy as np

import concourse.bacc as bacc
import concourse.bass as bass
import concourse.mybir as mybir
import concourse.tile as tile
from concourse.bass_utils import run_bass_kernel_spmd

F32 = mybir.dt.float32
BF16 = mybir.dt.bfloat16
I32 = mybir.dt.int32
ALU = mybir.AluOpType
ACTF = mybir.ActivationFunctionType

B, N, D, H = 8, 1024, 1024, 16
DH = D // H          # 64
P = 128
NT = N // P          # 8 token tiles
DT = D // P          # 8 feature tiles
VW = DH + 1          # 65: v block width incl ones column
PEN = 60.0           # additive logit penalty replacing the -1e9 mask

_CACHE = {}


def _poly_coeffs(table):
    """Exact degree-5 interpolating polynomial for table[x], x in 0..5.
    Returns a0..a5 (float)."""
    xs = np.arange(6, dtype=np.float64)
    V = np.vander(xs, 6, increasing=True)      # V @ a = table
    a = np.linalg.solve(V, table.astype(np.float64))
    return [float(v) for v in a]


def _build(coeffs, apply_gamma_beta):
    a0, a1, a2, a3, a4, a5 = coeffs
    nc = bacc.Bacc("TRN2", target_bir_lowering=False, debug=False)

    hid_d = nc.declare_dram_parameter("hid", [N, D], F32, False)
    adj_d = nc.declare_dram_parameter("adj", [N, N], I32, False)
    rp_d = nc.declare_dram_parameter("rp", [N, N], I32, False)
    wq_d = nc.declare_dram_parameter("wq", [D, D], F32, False)
    wk_d = nc.declare_dram_parameter("wk", [D, D], F32, False)
    wv_d = nc.declare_dram_parameter("wv", [D, D], F32, False)
    if apply_gamma_beta:
        gb_d = nc.declare_dram_parameter("gb", [2, D], F32, False)
    out_d = nc.declare_dram_parameter("out", [N, D], F32, True)

    with tile.TileContext(nc) as tc:
        with (
            tc.tile_pool(name="const", bufs=1) as cpool,
            tc.tile_pool(name="otp", bufs=1) as otpool,
            tc.tile_pool(name="qkvp", bufs=1) as qkvpool,
            tc.tile_pool(name="btp", bufs=1) as btpool,
        ):
            # ---- constants: identity matrices -------------------------------
            iota_t = cpool.tile([P, P], I32, name="iota_t")
            nc.gpsimd.iota(iota_t, pattern=[[1, P]], base=0, channel_multiplier=-1)
            i_f32 = cpool.tile([P, P], F32, name="i_f32")
            nc.vector.tensor_scalar(i_f32, iota_t, 0, None, op0=ALU.is_equal)
            i_bf = cpool.tile([P, P], BF16, name="i_bf")
            nc.vector.tensor_scalar(i_bf, iota_t, 0, None, op0=ALU.is_equal)

            # persistent big tensors
            qT = qkvpool.tile([P, DT * D], BF16, name="qT")    # [128, 8*1024]
            kT = qkvpool.tile([P, DT * D], BF16, name="kT")
            vv = [
                qkvpool.tile([P, H * VW], BF16, name=f"v{mt}", tag=f"v{mt}")
                for mt in range(NT)
            ]
            bT = btpool.tile([P, NT * N], BF16, name="bT")     # [128, 8*1024]
            outT = [
                otpool.tile([VW, N], BF16, name=f"outT{h}", tag=f"outT{h}")
                for h in range(H)
            ]

            with (
                tc.tile_pool(name="stage", bufs=1) as spool,
                tc.tile_pool(name="psumA", bufs=1, space="PSUM") as pspool,
            ):
                # ---- P2: load hid, build hidT (bf16, feature-major) ---------
                hidT = spool.tile([P, DT * N], BF16, name="hidT", tag="hidT")
                for nt in range(NT):
                    hst = spool.tile([P, D], F32, name="hst", tag="stage_f32",
                                     bufs=3)
                    nc.sync.dma_start(hst, hid_d[nt * P:(nt + 1) * P, :])
                    for g in range(2):
                        pt = pspool.tile([P, 4 * P], F32, name="pt_hid",
                                         tag="po", bufs=2)
                        for j in range(4):
                            db = g * 4 + j
                            nc.tensor.transpose(
                                pt[:, j * P:(j + 1) * P],
                                hst[:, db * P:(db + 1) * P], i_f32)
                        # scatter the 4 transposed d-blocks into hidT columns
                        dst = hidT.rearrange("p (d n) -> p d n", n=N)[
                            :, g * 4:(g + 1) * 4, nt * P:(nt + 1) * P]
                        src = pt.rearrange("p (d n) -> p d n", n=P)
                        nc.vector.tensor_copy(dst, src)

                # ---- P3: bias B = poly(rp) - PEN*(1-adj), then B^T ----------
                for nt in range(NT):
                    adj_sb = spool.tile([P, N], I32, name="adj_sb", tag="ints",
                                        bufs=4)
                    nc.sync.dma_start(adj_sb, adj_d[nt * P:(nt + 1) * P, :])
                    rp_sb = spool.tile([P, N], I32, name="rp_sb", tag="ints",
                                       bufs=4)
                    nc.sync.dma_start(rp_sb, rp_d[nt * P:(nt + 1) * P, :])

                    t = spool.tile([P, N], F32, name="ht", tag="ht", bufs=2)
                    nc.vector.tensor_scalar_mul(t, rp_sb, a5)
                    for ak in (a4, a3, a2, a1):
                        t2 = spool.tile([P, N], F32, name="ht2", tag="ht",
                                        bufs=2)
                        nc.vector.scalar_tensor_tensor(
                            t2, in0=t, scalar=ak, in1=rp_sb,
                            op0=ALU.add, op1=ALU.mult)
                        t = t2
                    z = spool.tile([P, N], F32, name="hz", tag="ht", bufs=2)
                    nc.vector.scalar_tensor_tensor(
                        z, in0=adj_sb, scalar=PEN, in1=t,
                        op0=ALU.mult, op1=ALU.add)
                    bn = spool.tile([P, N], BF16, name="bn", tag="bn", bufs=2)
                    nc.scalar.activation(bn, z, ACTF.Copy,
                                         bias=a0 - PEN, scale=1.0)
                    for g in range(2):
                        ptb = pspool.tile([P, 4 * P], BF16, name="pt_b",
                                          tag="po", bufs=2)
                        for j in range(4):
                            mb = g * 4 + j
                            nc.tensor.transpose(
                                ptb[:, j * P:(j + 1) * P],
                                bn[:, mb * P:(mb + 1) * P], i_bf)
                        dst = bT.rearrange("p (m n) -> p m n", n=N)[
                            :, g * 4:(g + 1) * 4, nt * P:(nt + 1) * P]
                        src = ptb.rearrange("p (m n) -> p m n", n=P)
                        nc.scalar.copy(dst, src)

                # ---- P1/P4: weights + QKV projections -----------------------
                wtiles = {}
                for pj, wd in (("q", wq_d), ("k", wk_d), ("v", wv_d)):
                    for dt_i in range(DT):
                        wst = spool.tile([P, D], F32, name="wst",
                                         tag="stage_f32", bufs=3)
                        nc.sync.dma_start(
                            wst, wd[dt_i * P:(dt_i + 1) * P, :])
                        wb = spool.tile([P, D], BF16, name=f"w_{pj}{dt_i}",
                                        tag=f"w{dt_i}", bufs=2)
                        if pj == "q":
                            nc.vector.tensor_scalar_mul(
                                wb, wst, float(1.0 / np.sqrt(np.float32(D))))
                        else:
                            nc.vector.tensor_copy(wb, wst)
                        wtiles[(pj, dt_i)] = wb

                for pj, dest in (("q", qT), ("k", kT)):
                    for ot in range(DT):
                        ps = pspool.tile([P, N], F32, name="ps_qk", tag="po",
                                         bufs=2)
                        for dt_i in range(DT):
                            for nh in range(2):
                                s = slice(nh * 512, (nh + 1) * 512)
                                nc.tensor.matmul(
                                    ps[:, s],
                                    wtiles[(pj, dt_i)][:, ot * P:(ot + 1) * P],
                                    hidT[:, dt_i * N + nh * 512:
                                         dt_i * N + (nh + 1) * 512],
                                    start=(dt_i == 0), stop=(dt_i == DT - 1))
                        nc.vector.tensor_copy(
                            dest[:, ot * N:(ot + 1) * N], ps)

                for mt in range(NT):
                    ps = pspool.tile([P, D], F32, name="ps_v", tag="po",
                                     bufs=2)
                    for dt_i in range(DT):
                        for oh in range(2):
                            s = slice(oh * 512, (oh + 1) * 512)
                            nc.tensor.matmul(
                                ps[:, s],
                                hidT[:, dt_i * N + mt * P:
                                     dt_i * N + (mt + 1) * P],
                                wtiles[("v", dt_i)][:, s],
                                start=(dt_i == 0), stop=(dt_i == DT - 1))
                    vt = vv[mt].rearrange("p (h c) -> p h c", c=VW)
                    nc.vector.tensor_copy(
                        vt[:, :, 0:DH],
                        ps.rearrange("p (h c) -> p h c", c=DH))
                    nc.vector.memset(vt[:, :, DH], 1.0)

                # ---- P5: attention ------------------------------------------
                for h in range(H):
                    av = pspool.tile([VW, N], F32, name="av", tag="av", bufs=2)
                    hr = (h % 2) * DH          # partition offset inside tile
                    hc = (h // 2) * N          # column offset of dout-tile
                    for mt in range(NT):
                        po = pspool.tile([P, N], F32, name="po_s", tag="po",
                                         bufs=2)
                        for nh in range(2):
                            s = slice(nh * 512, (nh + 1) * 512)
                            nc.tensor.matmul(
                                po[:, s], i_bf,
                                bT[:, mt * N + nh * 512:
                                   mt * N + (nh + 1) * 512],
                                start=True, stop=False)
                            nc.tensor.matmul(
                                po[:, s],
                                kT[hr:hr + DH,
                                   hc + mt * P:hc + (mt + 1) * P],
                                qT[hr:hr + DH, hc + nh * 512:
                                   hc + (nh + 1) * 512],
                                start=False, stop=True)
                        at = spool.tile([P, N], BF16, name="at", tag="at",
                                        bufs=3)
                        nc.scalar.activation(at, po, ACTF.Exp)
                        for nh in range(2):
                            s = slice(nh * 512, (nh + 1) * 512)
                            nc.tensor.matmul(
                                av[:, s],
                                vv[mt][:, h * VW:(h + 1) * VW],
                                at[:, s],
                                start=(mt == 0), stop=(mt == NT - 1))
                    nc.scalar.copy(outT[h], av)

            # ---- P6: transpose back, divide, relu, residual, layernorm ------
            with (
                tc.tile_pool(name="epi", bufs=1) as epool,
                tc.tile_pool(name="psumB", bufs=1, space="PSUM") as psb,
            ):
                if apply_gamma_beta:
                    gb = epool.tile([2, D], F32, name="gb")
                    nc.sync.dma_start(gb, gb_d[:, :])
                for nt in range(NT):
                    hid_r = epool.tile([P, D], F32, name="hid_r", tag="hid_r",
                                       bufs=2)
                    nc.sync.dma_start(hid_r, hid_d[nt * P:(nt + 1) * P, :])
                    tok = epool.tile([P, H * VW], F32, name="tok", tag="tok",
                                     bufs=2)
                    tokr = tok.rearrange("p (h c) -> p h c", c=VW)
                    for g in range(4):
                        pt2 = psb.tile([P, 4 * (VW + 1)], BF16, name="pt2",
                                       tag="pt", bufs=2)
                        for j in range(4):
                            h = g * 4 + j
                            nc.tensor.transpose(
                                pt2[:, j * (VW + 1):j * (VW + 1) + VW],
                                outT[h][:, nt * P:(nt + 1) * P],
                                i_bf[0:VW, 0:VW])
                        nc.scalar.copy(
                            tok.rearrange("p (h c) -> p h c", c=VW)[
                                :, g * 4:(g + 1) * 4, :],
                            pt2.rearrange("p (j c) -> p j c",
                                          c=VW + 1)[:, :, 0:VW])
                    den = epool.tile([P, H], F32, name="den", tag="den",
                                     bufs=2)
                    nc.vector.tensor_copy(den, tokr[:, :, DH])
                    rec = epool.tile([P, H], F32, name="rec", tag="rec",
                                     bufs=2)
                    nc.vector.reciprocal(rec, den)
                    res = epool.tile([P, D], F32, name="res", tag="res",
                                     bufs=2)
                    for h in range(H):
                        nc.vector.tensor_scalar(
                            res[:, h * DH:(h + 1) * DH],
                            tokr[:, h, 0:DH],
                            rec[:, h:h + 1], 0.0,
                            op0=ALU.mult, op1=ALU.max)
                    nc.vector.tensor_tensor(
                        out=res, in0=res, in1=hid_r, op=ALU.add)
                    st6 = epool.tile([P, 12], F32, name="st6", tag="st6",
                                     bufs=2)
                    nc.vector.bn_stats(st6[:, 0:6], res[:, 0:512])
                    nc.vector.bn_stats(st6[:, 6:12], res[:, 512:1024])
                    mv = epool.tile([P, 2], F32, name="mv", tag="mv", bufs=2)
                    nc.vector.bn_aggr(mv, st6)
                    ve = epool.tile([P, 1], F32, name="ve", tag="ve", bufs=2)
                    nc.vector.tensor_scalar_add(ve, mv[:, 1:2], 1e-5)
                    sd = epool.tile([P, 1], F32, name="sd", tag="sd", bufs=2)
                    nc.scalar.activation(sd, ve, ACTF.Sqrt)
                    rs = epool.tile([P, 1], F32, name="rs", tag="rs", bufs=2)
                    nc.vector.reciprocal(rs, sd)
                    nmr = epool.tile([P, 1], F32, name="nmr", tag="nmr",
                                     bufs=2)
                    nc.vector.scalar_tensor_tensor(
                        nmr, in0=mv[:, 0:1], scalar=-1.0, in1=rs,
                        op0=ALU.mult, op1=ALU.mult)
                    outf = epool.tile([P, D], F32, name="outf", tag="outf",
                                      bufs=2)
                    nc.scalar.activation(outf, res, ACTF.Identity,
                                         bias=nmr[:, 0:1], scale=rs[:, 0:1])
                    if apply_gamma_beta:
                        outg = epool.tile([P, D], F32, name="outg",
                                          tag="outg", bufs=2)
                        nc.vector.tensor_tensor(
                            out=outg, in0=outf,
                            in1=gb[0:1, :].partition_broadcast(P), op=ALU.mult)
                        nc.vector.tensor_tensor(
                            out=outg, in0=outg,
                            in1=gb[1:2, :].partition_broadcast(P), op=ALU.add)
                        outf = outg
                    nc.sync.dma_start(out_d[nt * P:(nt + 1) * P, :], outf)

    nc.compile()
    return nc


def kernel(hid, adj, relative_pos, Wq, Wk, Wv, rel_table, ln_gamma, ln_beta):
    hid = np.ascontiguousarray(np.asarray(hid, dtype=np.float32))
    adj = np.ascontiguousarray(np.asarray(adj, dtype=np.int32))
    rp = np.ascontiguousarray(np.asarray(relative_pos, dtype=np.int32))
    Wq = np.ascontiguousarray(np.asarray(Wq, dtype=np.float32))
    Wk = np.ascontiguousarray(np.asarray(Wk, dtype=np.float32))
    Wv = np.ascontiguousarray(np.asarray(Wv, dtype=np.float32))
    table = np.asarray(rel_table, dtype=np.float32).reshape(-1)
    gamma = np.asarray(ln_gamma, dtype=np.float32).reshape(-1)
    beta = np.asarray(ln_beta, dtype=np.float32).reshape(-1)

    coeffs = _poly_coeffs(table)
    trivial_gb = bool(np.all(gamma == 1.0) and np.all(beta == 0.0))

    key = (tuple(np.round(coeffs, 12)), trivial_gb)
    if key not in _CACHE:
        _CACHE[key] = _build(coeffs, not trivial_gb)
    nc = _CACHE[key]

    in_maps = []
    for b in range(B):
        m = {
            "hid": hid[b], "adj": adj[b], "rp": rp[b],
            "wq": Wq, "wk": Wk, "wv": Wv,
        }
        if not trivial_gb:
            m["gb"] = np.stack([gamma, beta])
        in_maps.append(m)

    res = run_bass_kernel_spmd(nc, in_maps, core_ids=list(range(B)))
    return np.stack([res.results[b]["out"] for b in range(B)])


if __name__ == "__main__":
    import reference
    inputs = {k: np.asarray(v) for k, v in reference.setup_inputs().items()}
    out = kernel(**inputs)
    exp = np.asarray(reference.reference(**inputs))
    err = np.linalg.norm((out - exp).ravel()) / np.linalg.norm(exp.ravel())
    print("Relative error:", err)
